# revision 1
# baseline (speedup 1.0000x reference)
"""Trainium2 Bass kernel for nn_Post_Prob (segment_reduce).

Reference computation, per image b (N=512 points, M=64*64=4096 pixels):
    dis[p, ij]  = (y_p - cood_i)^2 + (x_p - cood_j)^2
    min_dis[ij] = relu(min_p dis[p, ij])
    bg[ij]      = (0.15 * st_b)^2 / (min_dis + 1e-5)
    out         = softmax over the 513 rows of [-dis/128 ; -bg/128]

Sharding: data-parallel over the batch axis, 16 images -> 8 cores x 2.

All matmuls use float32r (1 cycle/row vs 4 for float32) with a K-dim
hi/lo split to recover fp32 accuracy: every fp32 operand row is split as
v = v_hi + v_lo (v_hi = 11 explicit mantissa bits, exactly representable
after the PE's ~13-bit fp32r rounding), and the four cross products are
carried by 4 K-rows per logical row. pts row-group 3 is all-ones, so the
phase-B rhs rows (12..15) = (c2_hi, c2_lo, L_hi, L_lo) fold both the c2
term and the softmax log-normalizer L = 128*ln(sum) into the matmul.

Per-core program (fully unrolled under TileContext), software-pipelined
at (image, column-quarter) granularity -- phase-A quads prefetched two
steps ahead, each quarter finalized one step before its output slices:
  Phase A (pixels on partitions, 32 chunks of [128 px, PMAX pts]):
    band-sparse: only points within BAND of each 16px strip participate
    (contributions beyond are below f32 resolution; softmax
    self-normalizes where the true min lies outside). K=16 matmul ->
    dis in PSUM (quads of 4 chunks, 2 banks); batched vector reduce-min
    -> per-pixel min; Exp + batched DVE reduce (or ACT accum_out during
    the image-0 ramp) -> per-pixel sum of exp.
  Finalize (per column quarter, 8 chunks): bg row segment;
    L = 128*ln(sum_pts + exp_bg); Veltkamp split into (L_hi, L_lo);
    PE-transpose of the [128, 8] column stats into pixel-major rows,
    DMA'd straight into rhs partitions 14/15.
  Phase B (points on partitions): K=16 matmul gives dis + L in PSUM;
    a single Exp(scale=-1/128) writes the final normalized
    probabilities; contiguous 512KB DMAs (16KB rows).
"""

import numpy as np

SIGMA = 8.0
C_SIZE = 512
STRIDE = 8
BG_RATIO = 0.15
EPS = 1e-5
B, N = 16, 512
C = C_SIZE // STRIDE  # 64
M = C * C  # 4096
NCORES = 8
BLOC = B // NCORES  # 2 images per core
INV = -1.0 / (2.0 * SIGMA * SIGMA)  # -1/128
CENTER = 256.0
NCHUNK = M // 128  # 32 pixel chunks in phase A
NPC = N // 128  # 4 point chunks in phase B
VC = float(3 << 20)  # Veltkamp constant: rounds to 0.25 grid for |L|<~1024
PMAX = 256  # padded per-strip candidate count for band-sparse phase A
BAND = 72.5  # y-distance beyond which exp(-dis/128) is invisible in f32
DUMMY_Y = 8000.0  # pad points: dis >= ~6e7 -> exp==0, never the min
VISPX = 60.0  # |dy| beyond which a point's output prob < ~6e-13 (write 0)
SORTPAD = 50.0  # allowed order-statistic fluctuation of sorted-chunk bounds

# With points sorted by y, output chunk pc covers y ~ [128pc-SORTPAD,
# 128(pc+1)+SORTPAD]; visible grid rows i = (y +- (VISPX+4))/8. Column
# quarters h (grid rows 16h..16h+15) outside that window hold values
# < ~6e-13 and are skipped -- the output buffer is pre-zeroed.
def _keep_table():
    keep = []
    for pc in range(4):
        lo_i = (128 * pc - SORTPAD - VISPX - 4.0) / 8.0
        hi_i = (128 * (pc + 1) + SORTPAD + VISPX - 4.0) / 8.0
        keep.append([bool(16 * h + 15 >= lo_i and 16 * h <= hi_i)
                     for h in range(4)])
    return keep

KEEP_B = _keep_table()

_CACHE = {}


def _split(v, bits=11):
    """v = hi + lo with hi keeping `bits` explicit mantissa bits."""
    u = np.ascontiguousarray(v, dtype=np.float32).view(np.uint32)
    hi = (u & np.uint32((0xFFFFFFFF << (23 - bits)) & 0xFFFFFFFF)).view(np.float32)
    lo = (v - hi).astype(np.float32)
    return hi, lo


def _host_consts():
    cood = (np.arange(0, C_SIZE, STRIDE, dtype=np.float32) + STRIDE / 2.0).astype(
        np.float32
    )
    cc = cood - np.float32(CENTER)  # centered pixel coords [64]
    ci = np.repeat(cc, C).astype(np.float32)  # i (y) varies slow over ij
    cj = np.tile(cc, C).astype(np.float32)  # j (x) varies fast
    c2 = (ci * ci + cj * cj).astype(np.float32)
    ones = np.ones(M, np.float32)
    zero = np.zeros(M, np.float32)

    # Pairing with pts16 row groups [phi, phi, plo, plo] (k=0..2) and
    # [1, 1, 1, 1] (k=3):
    ahi, alo = _split(-2.0 * ci)
    bhi, blo = _split(-2.0 * cj)
    chi, clo = _split(c2)
    const16 = np.stack(
        [
            ahi, alo, ahi, alo,       # k=0: * (y'hi, y'hi, y'lo, y'lo)
            bhi, blo, bhi, blo,       # k=1: * (x'hi, x'hi, x'lo, x'lo)
            ones, zero, ones, zero,   # k=2: * (r2hi, r2hi, r2lo, r2lo)
            chi, clo, zero, zero,     # k=3: * (1, 1, 1, 1); rows 14/15 get L
        ]
    ).astype(np.float32)  # [16, M]
    ident = np.eye(128, dtype=np.float32)
    return const16, ident


def _force_combined_act_table(arch="gen3"):
    """Restrict the activation-table chooser to natural_log_exp_and_others
    (contains exp+ln+copy) so the Exp...Ln...Exp sequence doesn't reload
    tables (~2.7us per reload on HW). Mutates the cached dict in place so
    ids stay aligned with act_info.json."""
    import concourse.hw_specs as hw_specs

    tabs = hw_specs.get_activation_tables(arch)
    keep = "natural_log_exp_and_others"
    if keep in tabs:
        for name, s in tabs.items():
            if name != keep:
                s.clear()


def _build(pmax=PMAX, dense_b=False, eprefix=None):
    import concourse.bacc as bacc
    import concourse.tile as tile
    import concourse.mybir as mybir

    _force_combined_act_table()

    f32 = mybir.dt.float32
    f32r = mybir.dt.float32r
    AF = mybir.ActivationFunctionType
    OP = mybir.AluOpType
    AX = mybir.AxisListType

    if eprefix is None:
        eprefix = pmax
    nc = bacc.Bacc("TRN2", target_bir_lowering=False, debug=False, num_devices=NCORES)

    pts_d = nc.dram_tensor("pts", [BLOC, 16, N], f32r, kind="ExternalInput")
    ptsA_d = nc.dram_tensor(
        "ptsA", [BLOC, 16, NCHUNK * pmax], f32r, kind="ExternalInput"
    )
    const16_d = nc.dram_tensor("const16", [16, M], f32r, kind="ExternalInput")
    sbg_d = nc.dram_tensor("sbg", [128, BLOC], f32, kind="ExternalInput")
    id_d = nc.dram_tensor("ident", [128, 128], f32, kind="ExternalInput")
    out_d = nc.dram_tensor("out", [BLOC, N + 1, M], f32, kind="ExternalOutput")

    with tile.TileContext(nc) as tc:
        with (
            tc.tile_pool(name="singles", bufs=1) as singles,
            tc.tile_pool(name="psA", bufs=4 if pmax <= 256 else 2, space="PSUM") as psA_pool,
            tc.tile_pool(name="psB", bufs=2, space="PSUM") as psB_pool,
            tc.tile_pool(name="expA", bufs=3 if pmax <= 256 else 2) as expA_pool,
            tc.tile_pool(name="cols", bufs=2) as cols_pool,
            tc.tile_pool(name="fin", bufs=2) as fin_pool,
            tc.tile_pool(name="rows", bufs=1) as rows_pool,
            tc.tile_pool(name="outb", bufs=6 if pmax <= 256 else 3) as outb_pool,
        ):
            # stage input loads: image-0 phase A needs const16 + ptsA[0]
            # first; everything else can trickle in behind the first chunks.
            const16_t = singles.tile([16, M], f32r)
            ptsA_tiles = []
            for b in range(BLOC):
                pa = singles.tile([16, NCHUNK * pmax], f32r, tag=f"ptsA{b}")
                ptsA_tiles.append(pa)
            # FIFO order matters on the DMA ring: image-0's critical-path
            # loads (const16/ptsA0 quarters, then id/sbg for fin(0,0), then
            # pts0 + rhsB0 for the first B slices) come before image-1 bulk.
            # first slice covers only chunks 0..7 (what fin(0,0) needs),
            # the rest arrives as one bulk DMA per tensor
            mq = M // 4
            aq = NCHUNK * pmax // 4
            nc.sync.dma_start(const16_t[:, 0:mq], const16_d[:, 0:mq])
            nc.sync.dma_start(ptsA_tiles[0][:, 0:aq], ptsA_d[0][:, 0:aq])
            nc.sync.dma_start(const16_t[:, mq:], const16_d[:, mq:])
            nc.sync.dma_start(ptsA_tiles[0][:, aq:], ptsA_d[0][:, aq:])
            id_t = singles.tile([128, 128], f32)
            nc.scalar.dma_start(id_t[:], id_d[:])
            sbg_t = singles.tile([128, BLOC], f32)
            nc.scalar.dma_start(sbg_t[:], sbg_d[:])
            pts_tiles = []
            rhsB_tiles = []
            for b in range(BLOC):
                pt = singles.tile([16, N], f32r, tag=f"pts{b}")
                pts_tiles.append(pt)
                rhsB = rows_pool.tile([16, M], f32r, tag=f"rhsB{b}")
                rhsB_tiles.append(rhsB)
            nc.sync.dma_start(pts_tiles[0][:], pts_d[0])
            nc.sync.dma_start(rhsB_tiles[0][0:14, :], const16_d[0:14, :])
            # image-1 bulk loads go via the ACT HWDGE ring so they don't
            # block the SP ring's path to the first output DMAs
            nc.scalar.dma_start(ptsA_tiles[1][:], ptsA_d[1])
            nc.scalar.dma_start(pts_tiles[1][:], pts_d[1])
            nc.scalar.dma_start(rhsB_tiles[1][0:14, :], const16_d[0:14, :])

            cols = {}
            for b in range(BLOC):
                mc = cols_pool.tile([128, NCHUNK], f32, tag=f"min{b}")
                sc = cols_pool.tile([128, NCHUNK], f32, tag=f"sum{b}")
                cols[b] = (mc, sc)

            # chunks per PSUM group: groups are capped at 2 banks so the
            # dense fallback (pmax=N) still fits PSUM alongside psB
            CG = 2

            def emit_A_group(b, g, act_sum=False):
                """Phase A, chunks CG*g..CG*g+CG-1: dis -> min + sum(exp).
                act_sum: half the chunks' sums via ACT accum_out instead of
                the batched DVE reduce -- shifts work DVE -> ACT."""
                min_cols, sum_cols = cols[b]
                ps = psA_pool.tile([128, CG, pmax], f32, tag="psA")
                for j in range(CG):
                    k = CG * g + j
                    nc.tensor.matmul(
                        ps[:, j, :],
                        const16_t[:, k * 128 : (k + 1) * 128],
                        ptsA_tiles[b][:, k * pmax : (k + 1) * pmax],
                        start=True,
                        stop=True,
                    )
                nc.vector.tensor_reduce(
                    min_cols[:, CG * g : CG * (g + 1)], ps[:, :, :eprefix],
                    axis=AX.X, op=OP.min,
                )
                ea = expA_pool.tile([128, CG, eprefix], f32, tag="expA")
                hc = CG // 2
                if act_sum:
                    # split the sum: first half via ACT accum, rest via DVE --
                    # balances the two engines during the image-0 ramp
                    for j in range(hc):
                        k = CG * g + j
                        nc.scalar.activation(
                            ea[:, j, :], ps[:, j, :eprefix], AF.Exp, bias=0.0,
                            scale=INV, accum_out=sum_cols[:, k : k + 1],
                        )
                    nc.scalar.activation(
                        ea[:, hc:CG, :], ps[:, hc:CG, :eprefix], AF.Exp,
                        bias=0.0, scale=INV,
                    )
                    nc.vector.tensor_reduce(
                        sum_cols[:, CG * g + hc : CG * (g + 1)], ea[:, hc:CG, :],
                        axis=AX.X, op=OP.add,
                    )
                else:
                    nc.scalar.activation(
                        ea[:], ps[:, :, :eprefix], AF.Exp, bias=0.0, scale=INV
                    )
                    # per-chunk in-place tensor_scalar (2x fp32 SBUF mode)
                    # with accum_out -- cheaper than the 1x tensor_reduce
                    for j in range(CG):
                        k = CG * g + j
                        nc.vector.tensor_scalar(
                            ea[:, j, :], ea[:, j, :], 1.0, 0.0, op0=OP.mult,
                            op1=OP.add, accum_out=sum_cols[:, k : k + 1],
                        )

            NH = NCHUNK // 4  # 8 chunks per output quarter
            def emit_fin_h(b, h):
                """Finalize pixel chunks 8h..8h+7: bg row segment +
                Veltkamp-split log-normalizer -> rhs cols h*1024..+1024."""
                min_cols, sum_cols = cols[b]
                rhsB = rhsB_tiles[b]
                mc = min_cols[:, NH * h : NH * (h + 1)]
                sc = sum_cols[:, NH * h : NH * (h + 1)]
                tmp = fin_pool.tile([128, NH], f32, tag="tmp")
                nc.vector.tensor_scalar(
                    tmp[:], mc, 0.0, EPS, op0=OP.max, op1=OP.add
                )
                rmin = fin_pool.tile([128, NH], f32, tag="rmin")
                nc.vector.reciprocal(rmin[:], tmp[:])
                bgd = fin_pool.tile([128, NH], f32, tag="bgd")
                nc.vector.tensor_scalar_mul(bgd[:], rmin[:], sbg_t[:, b : b + 1])
                ebg = fin_pool.tile([128, NH], f32, tag="ebg")
                nc.scalar.activation(ebg[:], bgd[:], AF.Exp, bias=0.0, scale=INV)
                stot = fin_pool.tile([128, NH], f32, tag="stot")
                nc.vector.tensor_tensor(stot[:], sc, ebg[:], op=OP.add)
                rsum = fin_pool.tile([128, NH], f32, tag="rsum")
                nc.vector.reciprocal(rsum[:], stot[:])
                # tin cols 0:8 -> L_hi, 8:16 -> L_lo, 16:24 -> bg probs
                tin = fin_pool.tile([128, 3 * NH], f32, tag="tin")
                nc.vector.tensor_tensor(
                    tin[:, 2 * NH : 3 * NH], ebg[:], rsum[:], op=OP.mult
                )
                lns = fin_pool.tile([128, NH], f32, tag="lns")
                nc.scalar.activation(lns[:], stot[:], AF.Ln, bias=0.0, scale=1.0)
                nc.vector.tensor_scalar_mul(lns[:], lns[:], 2.0 * SIGMA * SIGMA)
                nc.vector.tensor_scalar(
                    tin[:, 0:NH], lns[:], VC, -VC, op0=OP.add, op1=OP.add
                )
                nc.vector.tensor_tensor(
                    tin[:, NH : 2 * NH], lns[:], tin[:, 0:NH], op=OP.subtract
                )
                pst = psA_pool.tile([3 * NH, 128], f32, tag="psA")
                nc.tensor.transpose(pst[:], tin[:], id_t[:])
                tsb = fin_pool.tile([3 * NH, 128], f32, tag="tsb")
                nc.scalar.copy(tsb[:], pst[:])
                # L_hi/L_lo rows -> partitions 14/15 of the rhs tile cols
                nc.sync.dma_start(
                    rhsB[14:16, h * 1024 : (h + 1) * 1024]
                    .bitcast(f32)
                    .rearrange("p (k q) -> p k q", q=128),
                    tsb[0 : 2 * NH, :],
                )
                # background row segment of the output
                nc.sync.dma_start(
                    out_d[b, N, h * 1024 : (h + 1) * 1024].rearrange(
                        "(k q) -> k q", q=128
                    ),
                    tsb[2 * NH : 3 * NH, :],
                )

            def emit_B_slice(b, pc, h):
                """Phase B: one [128 pts, 1024 px] output slice."""
                psb = psB_pool.tile([128, 1024], f32, tag="psB")
                for s2 in range(2):
                    col0 = h * 1024 + s2 * 512
                    nc.tensor.matmul(
                        psb[:, s2 * 512 : (s2 + 1) * 512],
                        pts_tiles[b][:, pc * 128 : (pc + 1) * 128],
                        rhsB_tiles[b][:, col0 : col0 + 512],
                        start=True,
                        stop=True,
                    )
                outT = outb_pool.tile([128, 1024], f32, tag="outT")
                nc.scalar.activation(outT[:], psb[:], AF.Exp, bias=0.0, scale=INV)
                nc.sync.dma_start(
                    out_d[b, pc * 128 : (pc + 1) * 128, h * 1024 : (h + 1) * 1024],
                    outT[:],
                )

            # warm the activation table at t=0 (no data deps)
            warm = fin_pool.tile([1, 1], f32, tag="warm")
            nc.vector.memset(warm[:], 0.0)
            nc.scalar.activation(warm[:], warm[:], AF.Exp, bias=0.0, scale=1.0)

            # schedule: A0 | fin0 | {B0 x10 ~ A1 x16} | fin1 | B0 x6 | B1
            # (the last 6 B0 slices keep ACT busy while fin1's latency
            # chain + rhs row DMA resolve)
            # quarter-granularity pipeline over (image, column-quarter):
            # prefetch next quarter's phase-A quads, finalize this quarter,
            # then emit its 4 output slices.
            # alternate images so every ~5us another quarter finalizes and
            # the output-DMA stream never starves; thin quarters (2 kept
            # slices) go first, fat ones (3) last so the drain tail always
            # has parallel work
            seq = [(b, h) for h in (0, 3, 1, 2) for b in range(BLOC)]

            def emit_A_half(b, h, act_sum=False):
                # one half = 8 chunks = NH groups of CG
                g0 = 8 // CG * h
                for g in range(g0, g0 + 8 // CG):
                    emit_A_group(b, g, act_sum=act_sum)

            # software pipeline: quads two steps ahead, finalize one step
            # ahead of its B slices, so the fin latency chain never gates B.
            emit_A_half(*seq[0])
            emit_A_half(*seq[1])
            emit_fin_h(*seq[0])
            for idx, (b, h) in enumerate(seq):
                if idx + 2 < len(seq):
                    emit_A_half(*seq[idx + 2])
                if idx + 1 < len(seq):
                    emit_fin_h(*seq[idx + 1])
                for pc in range(NPC):
                    if dense_b or KEEP_B[pc][h]:
                        emit_B_slice(b, pc, h)

    nc.compile()
    return nc


def _get_nc(pmax=PMAX, dense_b=False, eprefix=None):
    key = ("nc", pmax, dense_b, eprefix)
    if key not in _CACHE:
        _CACHE[key] = _build(pmax, dense_b, eprefix)
        try:  # estimate before any run mutates the module
            from concourse.timeline_sim import TimelineSim

            _CACHE[key + ("est",)] = TimelineSim(
                _CACHE[key], trace=False
            ).simulate()
        except Exception:
            pass
    if "consts" not in _CACHE:
        _CACHE["consts"] = _host_consts()
    return _CACHE[key]


def _strip_index(y):
    """Per (image, strip) candidate point indices for the y-band."""
    return [
        [
            np.nonzero(
                (y[bb] >= 16 * k + 4 - BAND) & (y[bb] <= 16 * k + 12 + BAND)
            )[0]
            for k in range(NCHUNK)
        ]
        for bb in range(y.shape[0])
    ]


def _in_maps(points, st_sizes, pmax=PMAX):
    points = np.ascontiguousarray(np.asarray(points, dtype=np.float32))
    st_sizes = np.asarray(st_sizes, dtype=np.float32)
    const16, ident = _CACHE["consts"]
    in_maps = []
    for c in range(NCORES):
        sl = slice(BLOC * c, BLOC * (c + 1))
        p = points[sl]  # [BLOC, N, 2]
        # sort points by y so each 128-point output chunk has a bounded
        # y-range (enables the static column-quarter skip in phase B)
        p = np.stack([p[bb][np.argsort(p[bb, :, 1], kind="stable")]
                      for bb in range(BLOC)])
        xk = (p[..., 0] - np.float32(CENTER)).astype(np.float32)
        yk = (p[..., 1] - np.float32(CENTER)).astype(np.float32)
        r2 = (xk * xk + yk * yk).astype(np.float32)
        yhi, ylo = _split(yk)
        xhi, xlo = _split(xk)
        rhi, rlo = _split(r2)
        one = np.ones_like(xk)
        pts = np.ascontiguousarray(
            np.stack(
                [yhi, yhi, ylo, ylo, xhi, xhi, xlo, xlo, rhi, rhi, rlo, rlo,
                 one, one, one, one],
                axis=1,
            )
        )  # [BLOC, 16, N]

        # band-sparse phase-A point lists: per strip k (grid rows 2k, 2k+1,
        # y centers 16k+4 and 16k+12), points with y within BAND of the strip
        dyk = np.float32(DUMMY_Y)
        dummy = np.array(
            [dyk, dyk, 0, 0, 0, 0, 0, 0, dyk * dyk, dyk * dyk, 0, 0, 1, 1, 1, 1],
            np.float32,
        )
        ptsA = np.empty((BLOC, 16, NCHUNK, pmax), np.float32)
        ptsA[:] = dummy[None, :, None, None]
        for bb, strips in enumerate(_strip_index(p[..., 1])):
            for k, idx in enumerate(strips):
                assert len(idx) <= pmax
                ptsA[bb, :, k, : len(idx)] = pts[bb][:, idx]
        ptsA = np.ascontiguousarray(ptsA.reshape(BLOC, 16, NCHUNK * pmax))
        s = ((st_sizes[sl] * np.float32(BG_RATIO)) ** 2).astype(np.float32)
        sbg = np.ascontiguousarray(np.broadcast_to(s[None, :], (128, BLOC)))
        in_maps.append(
            {"pts": pts, "ptsA": ptsA, "const16": const16, "sbg": sbg,
             "ident": ident}
        )
    return in_maps


def _run(points, st_sizes, trace=False):
    from concourse.bass_utils import run_bass_kernel_spmd

    points = np.ascontiguousarray(np.asarray(points, dtype=np.float32))
    # band-sparse phase A assumes <= PMAX candidates per strip; fall back
    # to a dense (pmax=N) build for unusual point distributions
    mx = max(
        len(idx) for s in _strip_index(points[..., 1]) for idx in s
    )
    pmax = PMAX if mx <= PMAX else N
    # exp/sum/min only need the real-candidate prefix of each strip
    if mx <= 176:
        eprefix = 176
    elif mx <= 192:
        eprefix = 192
    else:
        eprefix = pmax
    # verify the static phase-B quarter-skip is safe for these points:
    # every sorted chunk's visible rows must lie inside its kept quarters
    ys = np.sort(points[..., 1], axis=1)  # [B, N]
    dense_b = False
    for bb in range(B):
        for pc in range(NPC):
            ylo, yhi = ys[bb, 128 * pc], ys[bb, 128 * pc + 127]
            ks = [h for h in range(4) if KEEP_B[pc][h]]
            # visible grid rows, clamped to the image (rows outside
            # [0, C) don't exist and need no coverage)
            need_lo = max((ylo - VISPX - 4.0) / 8.0, 0.0)
            need_hi = min((yhi + VISPX - 4.0) / 8.0, C - 1.0)
            if need_lo < 16 * ks[0] - 0.999 or need_hi > 16 * ks[-1] + 15.999:
                dense_b = True
    nc = _get_nc(pmax, dense_b, eprefix)
    _CACHE["last_est"] = _CACHE.get(("nc", pmax, dense_b, eprefix, "est"))
    res = run_bass_kernel_spmd(
        nc, _in_maps(points, st_sizes, pmax), core_ids=list(range(NCORES)),
        trace=trace,
    )
    out = np.concatenate([r["out"] for r in res.results], axis=0)
    # rows are in y-sorted order on device; scatter back to input order
    perm = np.argsort(points[..., 1], axis=1, kind="stable")  # [B, N]
    full = np.zeros_like(out)
    for bb in range(B):
        full[bb, perm[bb], :] = out[bb, :N, :]
        full[bb, N, :] = out[bb, N, :]
    return full, res


def kernel(points, st_sizes):
    out, _ = _run(points, st_sizes, trace=False)
    return out


def kernel_profiled(points, st_sizes):
    """Returns (out, BassKernelResults) with exec_time_ns populated."""
    return _run(points, st_sizes, trace=True)



# revision 26
# speedup vs baseline: 1.6900x; 1.6900x over previous
"""Trainium2 Bass kernel for nn_Post_Prob (segment_reduce).

Reference computation, per image b (N=512 points, M=64*64=4096 pixels):
    dis[p, ij]  = (y_p - cood_i)^2 + (x_p - cood_j)^2
    min_dis[ij] = relu(min_p dis[p, ij])
    bg[ij]      = (0.15 * st_b)^2 / (min_dis + 1e-5)
    out         = softmax over the 513 rows of [-dis/128 ; -bg/128]

Sharding: data-parallel over the batch axis, 16 images -> 8 cores x 2.

Slot-window design: points are y-sorted on the host, so point index
("slot") tracks y within ~27 px on this data.  Every per-pixel-chunk
computation then touches only a STATIC contiguous slot window:
  - matmul window: 256 slots centred on the chunk's y (padded pts array
    keeps it unclamped; fp32r needs >=256 output columns for 1 cyc/row)
  - work window 128 slots: exp (bf16) / per-quad sum + max reduces /
    per-quad broadcast scale run over [128 px, 128 slot] tiles
  - min_dis = -128*ln(max of exp) -- turns the PSUM min-reduce into a
    cheap SBUF max-reduce and two small fin ops
  - output: the scaled exp tile is PE-transposed (bf16, 1 cyc/row) to
    [slot, px] PSUM, copied to SBUF by the otherwise-idle GpSimd engine,
    and DMA'd with batched 3D APs (rows advance 16/chunk, 512B rows).
Values outside the windows are provably below ~2e-4 of the softmax
scale (the background term keeps Z >= 0.55); bf16 adds ~0.4% relative
noise -- both well inside the 2e-2 gate.
"""

import numpy as np

SIGMA = 8.0
C_SIZE = 512
STRIDE = 8
BG_RATIO = 0.15
EPS = 1e-5
B, N = 16, 512
C = C_SIZE // STRIDE  # 64
M = C * C  # 4096
NCORES = 8
BLOC = B // NCORES  # 2 images per core
INV = -1.0 / (2.0 * SIGMA * SIGMA)  # -1/128
CENTER = 256.0
NCHUNK = M // 128  # 32 pixel chunks (2 grid rows each)
PAD = 128  # slot padding each side of the 512 real slots
NPTS = N + 2 * PAD  # 768
DUMMY_Y = 8000.0
W0 = 128  # write/work window (slots)
T_WRITE = 36.0  # |dy| that must be inside the write window
T_SUM = 34.0  # |dy| that must be inside the work window

_CACHE = {}


def _split(v, bits=11):
    """v = hi + lo with hi keeping `bits` explicit mantissa bits."""
    u = np.ascontiguousarray(v, dtype=np.float32).view(np.uint32)
    hi = (u & np.uint32((0xFFFFFFFF << (23 - bits)) & 0xFFFFFFFF)).view(np.float32)
    lo = (v - hi).astype(np.float32)
    return hi, lo


def _host_consts():
    import ml_dtypes

    cood = (np.arange(0, C_SIZE, STRIDE, dtype=np.float32) + STRIDE / 2.0).astype(
        np.float32
    )
    cc = cood - np.float32(CENTER)
    ci = np.repeat(cc, C).astype(np.float32)  # i (y) varies slow over ij
    cj = np.tile(cc, C).astype(np.float32)  # j (x) varies fast
    c2 = (ci * ci + cj * cj).astype(np.float32)
    ones = np.ones(M, np.float32)
    zero = np.zeros(M, np.float32)
    ahi, alo = _split(-2.0 * ci)
    bhi, blo = _split(-2.0 * cj)
    chi, clo = _split(c2)
    const16 = np.stack(
        [
            ahi, alo, ahi, alo,       # k=0: * (y'hi, y'hi, y'lo, y'lo)
            bhi, blo, bhi, blo,       # k=1: * (x'hi, x'hi, x'lo, x'lo)
            ones, zero, ones, zero,   # k=2: * (r2hi, r2hi, r2lo, r2lo)
            chi, clo, zero, zero,     # k=3: * (1, 1, 1, 1)
        ]
    ).astype(np.float32)  # [16, M]
    ident = np.eye(128, dtype=np.float32)
    identb = np.eye(128).astype(ml_dtypes.bfloat16)
    return const16, ident, identb


def _windows():
    """Per-chunk static slot windows (padded coords for matmul/work)."""
    o_mm = [16 * k + 8 for k in range(NCHUNK)]  # padded start of 256-window
    ow = [min(max(16 * k + 8 - W0 // 2, 0), N - W0) for k in range(NCHUNK)]
    w_off = [ow[k] + PAD - o_mm[k] for k in range(NCHUNK)]  # within [0,256-W0]
    return o_mm, ow, w_off


def _quad_spans(w_off):
    """Per quad: (base, width) covering its 4 chunks' work windows."""
    spans = []
    for q in range(NCHUNK // 4):
        offs = w_off[4 * q : 4 * q + 4]
        lo, hi = min(offs), max(offs) + W0
        spans.append((lo, hi - lo))
    return spans


def _force_combined_act_table(arch="gen3"):
    """Keep exp+ln+copy in one activation table (avoids ~2.7us reloads)."""
    import concourse.hw_specs as hw_specs

    tabs = hw_specs.get_activation_tables(arch)
    keep = "natural_log_exp_and_others"
    if keep in tabs:
        for name, s in tabs.items():
            if name != keep:
                s.clear()


def _build(copy_eng=("act", "act"), ttmax_pool=False, sums_pool=()):
    import concourse.bacc as bacc
    import concourse.tile as tile
    import concourse.mybir as mybir
    from concourse.ap import AP

    _force_combined_act_table()

    f32 = mybir.dt.float32
    f32r = mybir.dt.float32r
    bf16 = mybir.dt.bfloat16
    AF = mybir.ActivationFunctionType
    OP = mybir.AluOpType
    AX = mybir.AxisListType

    o_mm, ow, w_off = _windows()
    spans = _quad_spans(w_off)
    NQ = NCHUNK // 4  # 8 quads per image

    def _ap(base_ap, extra_off, dims):
        """Custom-stride sub-AP of an existing AP (element units)."""
        pdim = [int(base_ap.ap[0][0]), int(base_ap.ap[0][1])]
        return AP(base_ap.tensor, base_ap.offset + extra_off, [pdim] + dims)

    nc = bacc.Bacc("TRN2", target_bir_lowering=False, debug=False, num_devices=NCORES)

    pts_d = nc.dram_tensor("pts", [BLOC, 16, NPTS], f32r, kind="ExternalInput")
    const16_d = nc.dram_tensor("const16", [16, M], f32r, kind="ExternalInput")
    sbg_d = nc.dram_tensor("sbg", [128, BLOC], f32, kind="ExternalInput")
    id_d = nc.dram_tensor("ident", [128, 128], f32, kind="ExternalInput")
    idb_d = nc.dram_tensor("identb", [128, 128], bf16, kind="ExternalInput")
    out_d = nc.dram_tensor("out", [BLOC, N + 1, M], f32, kind="ExternalOutput")

    EWM = max(w for _, w in spans)  # 176 on this layout

    with tile.TileContext(nc) as tc:
        with (
            tc.tile_pool(name="singles", bufs=1) as singles,
            tc.tile_pool(name="psA", bufs=2, space="PSUM") as psA_pool,
            tc.tile_pool(name="pst", bufs=2, space="PSUM") as pst_pool,
            tc.tile_pool(name="warmps", bufs=1, space="PSUM") as warm_pool,
            tc.tile_pool(name="ea", bufs=12) as ea_pool,
            tc.tile_pool(name="outb", bufs=3) as outb_pool,
            tc.tile_pool(name="cols", bufs=1) as cols_pool,
            tc.tile_pool(name="fin", bufs=2) as fin_pool,
        ):
            # input loads: first half of image 0 is on the critical path
            const16_t = singles.tile([16, M], f32r)
            pts_tiles = []
            for bb in range(BLOC):
                pt = singles.tile([16, NPTS], f32r, tag=f"pts{bb}")
                pts_tiles.append(pt)
            # first A-quad needs pts0 + const16 cols 0:512 only; ACT ring
            # carries just pts0 so the first exp isn't queued behind bulk,
            # Pool SWDGE takes the small constants (Pool idles early on)
            nc.sync.dma_start(const16_t[:, 0:512], const16_d[:, 0:512])
            nc.gpsimd.dma_start(pts_tiles[0][:], pts_d[0])
            idb_t = singles.tile([128, 128], bf16)
            nc.scalar.dma_start(idb_t[:], idb_d[:])
            nc.sync.dma_start(const16_t[:, 512:2048], const16_d[:, 512:2048])
            id_t = singles.tile([128, 128], f32)
            nc.gpsimd.dma_start(id_t[:], id_d[:])
            sbg_t = singles.tile([128, BLOC], f32)
            nc.gpsimd.dma_start(sbg_t[:], sbg_d[:])
            nc.scalar.dma_start(pts_tiles[1][:], pts_d[1])
            nc.sync.dma_start(const16_t[:, 2048:], const16_d[:, 2048:])

            cols = {}
            for bb in range(BLOC):
                cols[bb] = {}
                for nm, dt_ in (("mx", bf16), ("sm", f32), ("rz", f32),
                                ("bgp", f32)):
                    cols[bb][nm] = cols_pool.tile(
                        [128, NCHUNK], dt_, tag=f"{nm}{bb}", name=f"{nm}{bb}"
                    )

            eas = {}  # (bb, h) -> list of 4 ea tiles
            psts = {}  # (bb, h, g) -> pst tile

            def emit_A_mm(bb, h, qh):
                q = 4 * h + qh
                ps = psA_pool.tile([128, 4, 256], f32, tag="psA")
                for j in range(4):
                    k = 4 * q + j
                    nc.tensor.matmul(
                        ps[:, j, :],
                        const16_t[:, k * 128 : (k + 1) * 128],
                        pts_tiles[bb][:, o_mm[k] : o_mm[k] + 256],
                        start=True,
                        stop=True,
                    )
                return ps

            def emit_A_exp(bb, h, qh, ps):
                q = 4 * h + qh
                offs = w_off[4 * q : 4 * q + 4]
                dq = offs[1] - offs[0]  # 0 interior, -16 at the edges
                ea = ea_pool.tile([128, 4, W0], bf16, tag="ea", name="ea")
                src = _ap(ps[:], offs[0], [[256 + dq, 4], [1, W0]])
                nc.scalar.activation(
                    ea[:], src, AF.Exp, bias=0.0, scale=INV
                )
                eas[(bb, h)][qh] = ea
                return ea

            def emit_A_sums(bb, h, qh, ea):
                """Per-chunk sum-of-exp via 4x-mode tensor_scalar accum."""
                q = 4 * h + qh
                sm = cols[bb]["sm"]
                for j in range(4):
                    k = 4 * q + j
                    nc.vector.tensor_scalar(
                        ea[:, j, :], ea[:, j, :], 1.0, 0.0,
                        op0=OP.mult, op1=OP.add, accum_out=sm[:, k : k + 1],
                    )

            def emit_A_max(bb, h):
                """Half-level window max: two 2x tt-max halvings + reduce."""
                cc = cols[bb]
                qs = eas[(bb, h)]
                te1 = nc.vector
                t1 = fin_pool.tile([128, 4, 4, 64], bf16, tag="t1", name="t1")
                for qh in range(4):
                    ea = qs[qh]
                    te1.tensor_tensor(
                        t1[:, qh, :, :], ea[:, :, 0:64], ea[:, :, 64:128],
                        op=OP.max,
                    )
                te1.tensor_tensor(
                    t1[:, :, :, 0:32], t1[:, :, :, 0:32], t1[:, :, :, 32:64],
                    op=OP.max,
                )
                nc.vector.tensor_reduce(
                    cc["mx"][:, 16 * h : 16 * h + 16],
                    t1[:, :, :, 0:32], axis=AX.X, op=OP.max,
                )

            def emit_scale_quad(bb, h, qh):
                q = 4 * h + qh
                ea = eas[(bb, h)][qh]
                rz = cols[bb]["rz"]
                for j in range(4):
                    k = 4 * q + j
                    nc.vector.tensor_scalar_mul(
                        ea[:, j, :], ea[:, j, :], rz[:, k : k + 1]
                    )

            def emit_T_quad(bb, h, qh):
                ea = eas[(bb, h)][qh]
                g = qh // 2
                if qh % 2 == 0:
                    psts[(bb, h, g)] = pst_pool.tile([128, 8, W0], bf16, tag="pst", name="pst")
                ps = psts[(bb, h, g)]
                for j in range(4):
                    nc.tensor.transpose(
                        ps[:, 4 * (qh % 2) + j, :], ea[:, j, :], idb_t[:]
                    )

            def emit_copy(bb, h, g, eng):
                """copy a transposed 8-chunk group psum->sbuf (+f32 convert).
                GPSIMD cannot read PSUM, so only ACT/DVE are legal here."""
                outsb = outsbs[(bb, h)]
                ps = psts[(bb, h, g)]
                dst = outsb[:, 8 * g : 8 * g + 8, :]
                if eng == "act":
                    nc.scalar.copy(dst, ps[:])
                else:
                    nc.vector.tensor_copy(dst, ps[:])

            def emit_dma(bb, h, qh):
                """one quad's output rows: [W0 slots, 4 chunks, 128 px]."""
                outsb = outsbs[(bb, h)]
                gk0 = 16 * h + 4 * qh
                row0 = ow[gk0]
                drow = ow[gk0 + 1] - ow[gk0]
                bs = out_d[bb]
                dst = AP(
                    bs.tensor,
                    bs.offset + row0 * M + gk0 * 128,
                    [[M, W0], [drow * M + 128, 4], [1, 128]],
                )
                nc.sync.dma_start(dst, outsb[:, 4 * qh : 4 * qh + 4, :])

            def emit_fin_half(bb, h):
                """chunks 16h..16h+15: bg row segment + 1/Z per pixel."""
                cc = cols[bb]
                s = slice(16 * h, 16 * (h + 1))
                lnm = fin_pool.tile([128, 16], f32, tag="lnm")
                nc.scalar.activation(lnm[:], cc["mx"][:, s], AF.Ln, bias=0.0, scale=1.0)
                # min = -128 * clamp(ln(max), -90, 0); then *(-128) + eps
                nc.vector.tensor_scalar(
                    lnm[:], lnm[:], -90.0, 0.0, op0=OP.max, op1=OP.min
                )
                tmp = fin_pool.tile([128, 16], f32, tag="tmp")
                nc.vector.tensor_scalar(
                    tmp[:], lnm[:], -(2.0 * SIGMA * SIGMA), EPS,
                    op0=OP.mult, op1=OP.add,
                )
                rmin = fin_pool.tile([128, 16], f32, tag="rmin")
                nc.vector.reciprocal(rmin[:], tmp[:])
                bgd = fin_pool.tile([128, 16], f32, tag="bgd")
                nc.vector.tensor_scalar_mul(bgd[:], rmin[:], sbg_t[:, bb : bb + 1])
                ebg = fin_pool.tile([128, 16], f32, tag="ebg")
                nc.scalar.activation(ebg[:], bgd[:], AF.Exp, bias=0.0, scale=INV)
                stot = fin_pool.tile([128, 16], f32, tag="stot")
                nc.vector.tensor_tensor(stot[:], cc["sm"][:, s], ebg[:], op=OP.add)
                nc.vector.reciprocal(cc["rz"][:, s], stot[:])
                nc.vector.tensor_tensor(
                    cc["bgp"][:, s], ebg[:], cc["rz"][:, s], op=OP.mult
                )

            def emit_bg(bb):
                """background row: transpose bg probs, copy, one 16KB DMA."""
                ps = pst_pool.tile([128, 8, W0], bf16, tag="pst", name="pst")
                psf = ps[:32, 0:2, :].bitcast(f32)
                nc.tensor.transpose(psf, cols[bb]["bgp"][:], id_t[:])
                tsb = fin_pool.tile([32, 128], f32, tag="tsb")
                nc.scalar.copy(tsb[:], psf)
                nc.sync.dma_start(
                    out_d[bb, N, :].rearrange("(k q) -> k q", q=128), tsb[:]
                )

            # warm the activation table at t=0, and ramp the PE p-state
            # with a ~3us dummy matmul chain while the inputs stream in
            warm = fin_pool.tile([1, 1], f32, tag="warm")
            nc.vector.memset(warm[:], 0.0)
            nc.scalar.activation(warm[:], warm[:], AF.Exp, bias=0.0, scale=1.0)
            wsrc = singles.tile([16, 128], f32)
            nc.vector.memset(wsrc[:], 0.0)
            wps = warm_pool.tile([128, 128], f32)
            for _ in range(9):
                nc.tensor.matmul(wps[:], wsrc[:], wsrc[:, 0:128], start=True,
                                 stop=True)

            seq = [(bb, h) for bb in range(BLOC) for h in range(2)]
            outsbs = {}

            def emit_B_quad(bb, h, qh):
                emit_T_quad(bb, h, qh)
                if qh % 2 == 1:
                    emit_copy(bb, h, qh // 2, copy_eng[qh // 2])
                    emit_dma(bb, h, qh - 1)
                    emit_dma(bb, h, qh)

            prev = None
            for i, (bb, h) in enumerate(seq):
                eas[(bb, h)] = [None] * 4
                if prev is not None:
                    outsbs[prev] = outb_pool.tile(
                        [128, 16, W0], f32, tag="outsb", name="outsb"
                    )
                    for qh in range(4):
                        emit_scale_quad(*prev, qh)
                # PE: two matmul quads ahead of the B transposes
                pss = []
                for qh in range(6):
                    if qh < 4:
                        pss.append(emit_A_mm(bb, h, qh))
                        emit_A_exp(bb, h, qh, pss[qh])
                    if qh >= 2 and prev is not None:
                        emit_B_quad(*prev, qh - 2)
                for qh in range(4):
                    emit_A_sums(bb, h, qh, eas[(bb, h)][qh])
                emit_A_max(bb, h)
                emit_fin_half(bb, h)
                if h == 1:
                    emit_bg(bb)
                if prev is not None:
                    for g in (0, 1):
                        psts.pop((prev[0], prev[1], g), None)
                    eas.pop(prev)
                    outsbs.pop(prev)
                prev = (bb, h)

            # epilogue: B for the final half
            outsbs[prev] = outb_pool.tile(
                [128, 16, W0], f32, tag="outsb", name="outsb"
            )
            for qh in range(4):
                emit_scale_quad(*prev, qh)
            for qh in range(4):
                emit_B_quad(*prev, qh)

    nc.compile()
    return nc


def _get_nc(cfg=()):
    key = ("nc",) + tuple(cfg)
    if key not in _CACHE:
        _CACHE[key] = _build(*cfg) if cfg else _build()
        try:
            from concourse.timeline_sim import TimelineSim

            _CACHE[("nc", "est")] = TimelineSim(_CACHE[key], trace=False).simulate()
        except Exception:
            pass
    if "consts" not in _CACHE:
        _CACHE["consts"] = _host_consts()
    return _CACHE[key]


def _coverage_ok(points):
    """Check the static windows cover this data's candidate slots."""
    _, ow, _ = _windows()
    for bb in range(points.shape[0]):
        y = np.sort(points[bb, :, 1])
        slots = np.arange(N)
        for k in range(NCHUNK):
            c = 16 * k + 8
            for t in (T_WRITE, T_SUM):
                m = np.abs(y - c) <= t
                if m.any():
                    lo, hi = slots[m].min(), slots[m].max()
                    if lo < ow[k] or hi >= ow[k] + W0:
                        return False
    return True


def _in_maps(points, st_sizes):
    points = np.ascontiguousarray(np.asarray(points, dtype=np.float32))
    st_sizes = np.asarray(st_sizes, dtype=np.float32)
    const16, ident, identb = _CACHE["consts"]
    in_maps = []
    for c in range(NCORES):
        sl = slice(BLOC * c, BLOC * (c + 1))
        p = points[sl]  # [BLOC, N, 2]
        p = np.stack(
            [p[bb][np.argsort(p[bb, :, 1], kind="stable")] for bb in range(BLOC)]
        )
        # pad with far-away dummies so slot windows never clamp
        pp = np.empty((BLOC, NPTS, 2), np.float32)
        pp[:, :, 0] = 0.0
        pp[:, :, 1] = DUMMY_Y
        pp[:, PAD : PAD + N] = p
        xk = (pp[..., 0] - np.float32(CENTER)).astype(np.float32)
        yk = (pp[..., 1] - np.float32(CENTER)).astype(np.float32)
        r2 = (xk * xk + yk * yk).astype(np.float32)
        yhi, ylo = _split(yk)
        xhi, xlo = _split(xk)
        rhi, rlo = _split(r2)
        one = np.ones_like(xk)
        pts = np.ascontiguousarray(
            np.stack(
                [yhi, yhi, ylo, ylo, xhi, xhi, xlo, xlo, rhi, rhi, rlo, rlo,
                 one, one, one, one],
                axis=1,
            )
        )  # [BLOC, 16, NPTS]
        s = ((st_sizes[sl] * np.float32(BG_RATIO)) ** 2).astype(np.float32)
        sbg = np.ascontiguousarray(np.broadcast_to(s[None, :], (128, BLOC)))
        in_maps.append(
            {"pts": pts, "const16": const16, "sbg": sbg, "ident": ident,
             "identb": identb}
        )
    return in_maps


def _numpy_fallback(points, st_sizes):
    """Dense host computation for pathological point layouts."""
    cood = np.arange(0, C_SIZE, STRIDE, dtype=np.float64) + STRIDE / 2.0
    out = np.empty((B, N + 1, M), np.float32)
    for bb in range(B):
        x = points[bb, :, 0].astype(np.float64)
        y = points[bb, :, 1].astype(np.float64)
        xd = (x[:, None] - cood) ** 2
        yd = (y[:, None] - cood) ** 2
        dis = (yd[:, :, None] + xd[:, None, :]).reshape(N, M)
        mind = np.clip(dis.min(axis=0), 0.0, None)
        bg = (float(st_sizes[bb]) * BG_RATIO) ** 2 / (mind + EPS)
        logits = np.concatenate([dis, bg[None]], axis=0) * (-1.0 / 128.0)
        e = np.exp(logits - logits.max(axis=0))
        out[bb] = (e / e.sum(axis=0)).astype(np.float32)
    return out


def _run(points, st_sizes, trace=False):
    from concourse.bass_utils import run_bass_kernel_spmd

    points = np.ascontiguousarray(np.asarray(points, dtype=np.float32))
    if not _coverage_ok(points):
        return _numpy_fallback(points, np.asarray(st_sizes)), None
    nc = _get_nc()
    _CACHE["last_est"] = _CACHE.get(("nc", "est"))
    res = run_bass_kernel_spmd(
        nc, _in_maps(points, st_sizes), core_ids=list(range(NCORES)),
        trace=trace,
    )
    out = np.concatenate([r["out"] for r in res.results], axis=0)
    # rows are in y-sorted order on device; scatter back to input order
    perm = np.argsort(points[..., 1], axis=1, kind="stable")  # [B, N]
    full = np.zeros_like(out)
    for bb in range(B):
        full[bb, perm[bb], :] = out[bb, :N, :]
        full[bb, N, :] = out[bb, N, :]
    return full, res


def kernel(points, st_sizes):
    out, _ = _run(points, st_sizes, trace=False)
    return out


def kernel_profiled(points, st_sizes):
    """Returns (out, BassKernelResults) with exec_time_ns populated."""
    return _run(points, st_sizes, trace=True)


# revision 38
# speedup vs baseline: 1.7396x; 1.0294x over previous
"""Trainium2 Bass kernel for nn_Post_Prob (segment_reduce).

Reference computation, per image b (N=512 points, M=64*64=4096 pixels):
    dis[p, ij]  = (y_p - cood_i)^2 + (x_p - cood_j)^2
    min_dis[ij] = relu(min_p dis[p, ij])
    bg[ij]      = (0.15 * st_b)^2 / (min_dis + 1e-5)
    out         = softmax over the 513 rows of [-dis/128 ; -bg/128]

Sharding: data-parallel over the batch axis, 16 images -> 8 cores x 2.

Slot-window design: points are y-sorted on the host, so point index
("slot") tracks y within ~27 px on this data.  Every per-pixel-chunk
computation then touches only a STATIC contiguous slot window:
  - matmul window: 256 slots centred on the chunk's y (padded pts array
    keeps it unclamped; fp32r needs >=256 output columns for 1 cyc/row)
  - work window 128 slots: exp (bf16) / per-quad sum + max reduces /
    per-quad broadcast scale run over [128 px, 128 slot] tiles
  - min_dis = -128*ln(max of exp) -- turns the PSUM min-reduce into a
    cheap SBUF max-reduce and two small fin ops
  - output: the scaled exp tile is PE-transposed (bf16, 1 cyc/row) to
    [slot, px] PSUM, copied to SBUF by the otherwise-idle GpSimd engine,
    and DMA'd with batched 3D APs (rows advance 16/chunk, 512B rows).
Values outside the windows are provably below ~2e-4 of the softmax
scale (the background term keeps Z >= 0.55); bf16 adds ~0.4% relative
noise -- both well inside the 2e-2 gate.
"""

import numpy as np

SIGMA = 8.0
C_SIZE = 512
STRIDE = 8
BG_RATIO = 0.15
EPS = 1e-5
B, N = 16, 512
C = C_SIZE // STRIDE  # 64
M = C * C  # 4096
NCORES = 8
BLOC = B // NCORES  # 2 images per core
INV = -1.0 / (2.0 * SIGMA * SIGMA)  # -1/128
CENTER = 256.0
NCHUNK = M // 128  # 32 pixel chunks (2 grid rows each)
PAD = 128  # slot padding each side of the 512 real slots
NPTS = N + 2 * PAD  # 768
DUMMY_Y = 8000.0
W0 = 128  # write/work window (slots)
T_WRITE = 36.0  # |dy| that must be inside the write window
T_SUM = 34.0  # |dy| that must be inside the work window

_CACHE = {}


def _split(v, bits=11):
    """v = hi + lo with hi keeping `bits` explicit mantissa bits."""
    u = np.ascontiguousarray(v, dtype=np.float32).view(np.uint32)
    hi = (u & np.uint32((0xFFFFFFFF << (23 - bits)) & 0xFFFFFFFF)).view(np.float32)
    lo = (v - hi).astype(np.float32)
    return hi, lo


def _host_consts():
    import ml_dtypes

    cood = (np.arange(0, C_SIZE, STRIDE, dtype=np.float32) + STRIDE / 2.0).astype(
        np.float32
    )
    cc = cood - np.float32(CENTER)
    ci = np.repeat(cc, C).astype(np.float32)  # i (y) varies slow over ij
    cj = np.tile(cc, C).astype(np.float32)  # j (x) varies fast
    c2 = (ci * ci + cj * cj).astype(np.float32)
    ones = np.ones(M, np.float32)
    zero = np.zeros(M, np.float32)
    ahi, alo = _split(-2.0 * ci)
    bhi, blo = _split(-2.0 * cj)
    chi, clo = _split(c2)
    const16 = np.stack(
        [
            ahi, alo, ahi, alo,       # k=0: * (y'hi, y'hi, y'lo, y'lo)
            bhi, blo, bhi, blo,       # k=1: * (x'hi, x'hi, x'lo, x'lo)
            ones, zero, ones, zero,   # k=2: * (r2hi, r2hi, r2lo, r2lo)
            chi, clo, zero, zero,     # k=3: * (1, 1, 1, 1)
        ]
    ).astype(np.float32)  # [16, M]
    ident = np.eye(128, dtype=np.float32)
    identb = np.eye(128).astype(ml_dtypes.bfloat16)
    return const16, ident, identb


def _windows(ow=None):
    """Per-chunk static slot windows (padded coords for matmul/work)."""
    o_mm = [16 * k + 8 for k in range(NCHUNK)]  # padded start of 256-window
    if ow is None:
        ow = [min(max(16 * k + 8 - W0 // 2, 0), N - W0) for k in range(NCHUNK)]
    w_off = [ow[k] + PAD - o_mm[k] for k in range(NCHUNK)]  # within [0,256-W0]
    return o_mm, list(ow), w_off


def _fit_windows(points):
    """Choose per-chunk window starts covering this data's candidate
    slots.  Returns (ow tuple, grouping) or None if infeasible.  Prefers
    the canonical stride-16 layout; falls back to per-chunk shifts with
    degraded DMA batching."""
    canon = [min(max(16 * k + 8 - W0 // 2, 0), N - W0) for k in range(NCHUNK)]
    lo = [0] * NCHUNK
    hi = [-1] * NCHUNK
    for bb in range(points.shape[0]):
        y = np.sort(points[bb, :, 1])
        for k in range(NCHUNK):
            c = 16 * k + 8
            m = np.abs(y - c) <= T_WRITE
            if m.any():
                idx = np.nonzero(m)[0]
                lo[k] = min(lo[k], int(idx[0])) if hi[k] >= 0 else int(idx[0])
                hi[k] = max(hi[k], int(idx[-1]))
    ow = []
    for k in range(NCHUNK):
        if hi[k] < 0:
            ow.append(canon[k])
            continue
        if hi[k] - lo[k] >= W0:
            return None  # window cannot cover; numpy fallback
        o = min(max(canon[k], hi[k] - W0 + 1), lo[k])
        o = min(max(o, 0), N - W0)
        if o > lo[k] or o + W0 <= hi[k]:
            return None
        # matmul 256-window must contain the work window
        if not (0 <= o + PAD - (16 * k + 8) <= 256 - W0):
            return None
        ow.append(o)
    return tuple(ow)


def _dma_groups(ow):
    """Split each half's 16 chunks into maximal uniform-stride runs."""
    groups = {}
    for h in (0, 1):
        k0, k1 = 16 * h, 16 * h + 16
        runs = []
        s = k0
        while s < k1:
            cap = k0 + 8 if s < k0 + 8 else k1  # break at copy boundary
            e = s + 1
            if e < cap:
                d = ow[e] - ow[e - 1]
                while e < cap and ow[e] - ow[e - 1] == d:
                    e += 1
            runs.append((s, e - s))
            s = e
        groups[h] = runs
    return groups


def _quad_spans(w_off):
    """Per quad: (base, width) covering its 4 chunks' work windows."""
    spans = []
    for q in range(NCHUNK // 4):
        offs = w_off[4 * q : 4 * q + 4]
        lo, hi = min(offs), max(offs) + W0
        spans.append((lo, hi - lo))
    return spans


def _force_combined_act_table(arch="gen3"):
    """Keep exp+ln+copy in one activation table (avoids ~2.7us reloads)."""
    import concourse.hw_specs as hw_specs

    tabs = hw_specs.get_activation_tables(arch)
    keep = "natural_log_exp_and_others"
    if keep in tabs:
        for name, s in tabs.items():
            if name != keep:
                s.clear()


def _build(copy_eng=("act", "act"), ow_t=None):
    import concourse.bacc as bacc
    import concourse.tile as tile
    import concourse.mybir as mybir
    from concourse.ap import AP

    _force_combined_act_table()

    f32 = mybir.dt.float32
    f32r = mybir.dt.float32r
    bf16 = mybir.dt.bfloat16
    AF = mybir.ActivationFunctionType
    OP = mybir.AluOpType
    AX = mybir.AxisListType

    o_mm, ow, w_off = _windows(ow_t)
    spans = _quad_spans(w_off)
    dgroups = _dma_groups(ow)
    NQ = NCHUNK // 4  # 8 quads per image

    def _ap(base_ap, extra_off, dims):
        """Custom-stride sub-AP of an existing AP (element units)."""
        pdim = [int(base_ap.ap[0][0]), int(base_ap.ap[0][1])]
        return AP(base_ap.tensor, base_ap.offset + extra_off, [pdim] + dims)

    nc = bacc.Bacc("TRN2", target_bir_lowering=False, debug=False, num_devices=NCORES)

    pts_d = nc.dram_tensor("pts", [BLOC, 16, NPTS], f32r, kind="ExternalInput")
    const16_d = nc.dram_tensor("const16", [16, M], f32r, kind="ExternalInput")
    sbg_d = nc.dram_tensor("sbg", [128, BLOC], f32, kind="ExternalInput")
    id_d = nc.dram_tensor("ident", [128, 128], f32, kind="ExternalInput")
    idb_d = nc.dram_tensor("identb", [128, 128], bf16, kind="ExternalInput")
    out_d = nc.dram_tensor("out", [BLOC, N + 1, M], f32, kind="ExternalOutput")

    EWM = max(w for _, w in spans)  # 176 on this layout

    with tile.TileContext(nc) as tc:
        with (
            tc.tile_pool(name="singles", bufs=1) as singles,
            tc.tile_pool(name="psA", bufs=2, space="PSUM") as psA_pool,
            tc.tile_pool(name="pst", bufs=2, space="PSUM") as pst_pool,
            tc.tile_pool(name="warmps", bufs=1, space="PSUM") as warm_pool,
            tc.tile_pool(name="ea", bufs=12) as ea_pool,
            tc.tile_pool(name="outb", bufs=3) as outb_pool,
            tc.tile_pool(name="cols", bufs=1) as cols_pool,
            tc.tile_pool(name="fin", bufs=2) as fin_pool,
        ):
            # input loads: first half of image 0 is on the critical path
            const16_t = singles.tile([16, M], f32r)
            pts_tiles = []
            for bb in range(BLOC):
                pt = singles.tile([16, NPTS], f32r, tag=f"pts{bb}")
                pts_tiles.append(pt)
            # first A-quad needs pts0 + const16 cols 0:512 only; ACT ring
            # carries just pts0 so the first exp isn't queued behind bulk,
            # Pool SWDGE takes the small constants (Pool idles early on)
            nc.sync.dma_start(const16_t[:, 0:512], const16_d[:, 0:512])
            nc.gpsimd.dma_start(pts_tiles[0][:], pts_d[0])
            idb_t = singles.tile([128, 128], bf16)
            nc.scalar.dma_start(idb_t[:], idb_d[:])
            nc.sync.dma_start(const16_t[:, 512:2048], const16_d[:, 512:2048])
            id_t = singles.tile([128, 128], f32)
            nc.gpsimd.dma_start(id_t[:], id_d[:])
            sbg_t = singles.tile([128, BLOC], f32)
            nc.gpsimd.dma_start(sbg_t[:], sbg_d[:])
            nc.scalar.dma_start(pts_tiles[1][:], pts_d[1])
            nc.sync.dma_start(const16_t[:, 2048:], const16_d[:, 2048:])

            cols = {}
            for bb in range(BLOC):
                cols[bb] = {}
                for nm, dt_ in (("mx", bf16), ("sm", f32), ("rz", f32),
                                ("bgp", f32)):
                    cols[bb][nm] = cols_pool.tile(
                        [128, NCHUNK], dt_, tag=f"{nm}{bb}", name=f"{nm}{bb}"
                    )

            eas = {}  # (bb, h) -> list of 4 ea tiles
            psts = {}  # (bb, h, g) -> pst tile

            def emit_A_mm(bb, h, qh):
                q = 4 * h + qh
                ps = psA_pool.tile([128, 4, 256], f32, tag="psA")
                for j in range(4):
                    k = 4 * q + j
                    nc.tensor.matmul(
                        ps[:, j, :],
                        const16_t[:, k * 128 : (k + 1) * 128],
                        pts_tiles[bb][:, o_mm[k] : o_mm[k] + 256],
                        start=True,
                        stop=True,
                    )
                return ps

            def emit_A_exp(bb, h, qh, ps):
                q = 4 * h + qh
                offs = w_off[4 * q : 4 * q + 4]
                dq = offs[1] - offs[0]  # 0 interior, -16 at the edges
                ea = ea_pool.tile([128, 4, W0], bf16, tag="ea", name="ea")
                src = _ap(ps[:], offs[0], [[256 + dq, 4], [1, W0]])
                nc.scalar.activation(
                    ea[:], src, AF.Exp, bias=0.0, scale=INV
                )
                eas[(bb, h)][qh] = ea
                return ea

            def emit_A_sums(bb, h, qh, ea):
                """Per-chunk sum-of-exp via 4x-mode tensor_scalar accum."""
                q = 4 * h + qh
                sm = cols[bb]["sm"]
                for j in range(4):
                    k = 4 * q + j
                    nc.vector.tensor_scalar(
                        ea[:, j, :], ea[:, j, :], 1.0, 0.0,
                        op0=OP.mult, op1=OP.add, accum_out=sm[:, k : k + 1],
                    )

            def emit_A_max(bb, h):
                """Half-level window max: two 2x tt-max halvings + reduce."""
                cc = cols[bb]
                qs = eas[(bb, h)]
                hw_, qw = W0 // 2, W0 // 4
                t1 = fin_pool.tile([128, 4, 4, hw_], bf16, tag="t1", name="t1")
                for qh in range(4):
                    ea = qs[qh]
                    nc.vector.tensor_tensor(
                        t1[:, qh, :, :], ea[:, :, 0:hw_], ea[:, :, hw_ : 2 * hw_],
                        op=OP.max,
                    )
                nc.vector.tensor_tensor(
                    t1[:, :, :, 0:qw], t1[:, :, :, 0:qw],
                    t1[:, :, :, qw : 2 * qw], op=OP.max,
                )
                nc.vector.tensor_reduce(
                    cc["mx"][:, 16 * h : 16 * h + 16],
                    t1[:, :, :, 0:qw], axis=AX.X, op=OP.max,
                )

            def emit_scale_quad(bb, h, qh):
                q = 4 * h + qh
                ea = eas[(bb, h)][qh]
                rz = cols[bb]["rz"]
                for j in range(4):
                    k = 4 * q + j
                    nc.vector.tensor_scalar_mul(
                        ea[:, j, :], ea[:, j, :], rz[:, k : k + 1]
                    )

            def emit_T_quad(bb, h, qh):
                ea = eas[(bb, h)][qh]
                g = qh // 2
                if qh % 2 == 0:
                    psts[(bb, h, g)] = pst_pool.tile([128, 8, 128], bf16, tag="pst", name="pst")
                ps = psts[(bb, h, g)]
                for j in range(4):
                    nc.tensor.transpose(
                        ps[:W0, 4 * (qh % 2) + j, :], ea[:, j, :], idb_t[:]
                    )

            def emit_copy(bb, h, g, eng):
                """copy a transposed 8-chunk group psum->sbuf (+f32 convert).
                GPSIMD cannot read PSUM, so only ACT/DVE are legal here."""
                outsb = outsbs[(bb, h)]
                ps = psts[(bb, h, g)]
                dst = outsb[:W0, 8 * g : 8 * g + 8, :]
                srcg = ps[:W0, :, :]
                if eng == "act":
                    nc.scalar.copy(dst, srcg)
                else:
                    nc.vector.tensor_copy(dst, srcg)

            def emit_dma(bb, h, gk0, gn):
                """output rows for gn chunks: [W0 slots, gn chunks, 128 px]."""
                outsb = outsbs[(bb, h)]
                row0 = ow[gk0]
                drow = ow[gk0 + 1] - ow[gk0] if gn > 1 else 0
                bs = out_d[bb]
                dst = AP(
                    bs.tensor,
                    bs.offset + row0 * M + gk0 * 128,
                    [[M, W0], [drow * M + 128, gn], [1, 128]],
                )
                c0 = gk0 - 16 * h
                nc.sync.dma_start(dst, outsb[:W0, c0 : c0 + gn, :])

            def emit_fin_half(bb, h):
                """chunks 16h..16h+15: bg row segment + 1/Z per pixel."""
                cc = cols[bb]
                s = slice(16 * h, 16 * (h + 1))
                lnm = fin_pool.tile([128, 16], f32, tag="lnm")
                nc.scalar.activation(lnm[:], cc["mx"][:, s], AF.Ln, bias=0.0, scale=1.0)
                # min = -128 * clamp(ln(max), -90, 0); then *(-128) + eps
                nc.vector.tensor_scalar(
                    lnm[:], lnm[:], -90.0, 0.0, op0=OP.max, op1=OP.min
                )
                tmp = fin_pool.tile([128, 16], f32, tag="tmp")
                nc.vector.tensor_scalar(
                    tmp[:], lnm[:], -(2.0 * SIGMA * SIGMA), EPS,
                    op0=OP.mult, op1=OP.add,
                )
                rmin = fin_pool.tile([128, 16], f32, tag="rmin")
                nc.vector.reciprocal(rmin[:], tmp[:])
                bgd = fin_pool.tile([128, 16], f32, tag="bgd")
                nc.vector.tensor_scalar_mul(bgd[:], rmin[:], sbg_t[:, bb : bb + 1])
                ebg = fin_pool.tile([128, 16], f32, tag="ebg")
                nc.scalar.activation(ebg[:], bgd[:], AF.Exp, bias=0.0, scale=INV)
                stot = fin_pool.tile([128, 16], f32, tag="stot")
                nc.vector.tensor_tensor(stot[:], cc["sm"][:, s], ebg[:], op=OP.add)
                nc.vector.reciprocal(cc["rz"][:, s], stot[:])
                nc.vector.tensor_tensor(
                    cc["bgp"][:, s], ebg[:], cc["rz"][:, s], op=OP.mult
                )

            def emit_bg(bb):
                """background row: transpose bg probs, copy, one 16KB DMA."""
                ps = pst_pool.tile([128, 8, 128], bf16, tag="pst", name="pst")
                psf = ps[:32, 0:2, :].bitcast(f32)
                nc.tensor.transpose(psf, cols[bb]["bgp"][:], id_t[:])
                tsb = fin_pool.tile([32, 128], f32, tag="tsb")
                nc.scalar.copy(tsb[:], psf)
                nc.sync.dma_start(
                    out_d[bb, N, :].rearrange("(k q) -> k q", q=128), tsb[:]
                )

            # warm the activation table at t=0, and ramp the PE p-state
            # with a ~3us dummy matmul chain while the inputs stream in
            warm = fin_pool.tile([1, 1], f32, tag="warm")
            nc.vector.memset(warm[:], 0.0)
            nc.scalar.activation(warm[:], warm[:], AF.Exp, bias=0.0, scale=1.0)
            wsrc = singles.tile([16, 128], f32)
            nc.vector.memset(wsrc[:], 0.0)
            wps = warm_pool.tile([128, 128], f32)
            for _ in range(6):
                nc.tensor.matmul(wps[:], wsrc[:], wsrc[:, 0:128], start=True,
                                 stop=True)

            seq = [(bb, h) for bb in range(BLOC) for h in range(2)]
            outsbs = {}

            def emit_B_quad(bb, h, qh):
                emit_T_quad(bb, h, qh)

            def emit_B_drain(bb, h):
                """copies + DMAs; emitted late so ACT exps aren't blocked."""
                hi = 2 * (2 * bb + h)
                runs = dgroups[h]
                emit_copy(bb, h, 0, copy_eng[hi % len(copy_eng)])
                for gk0, gn in runs:
                    if gk0 + gn <= 16 * h + 8:
                        emit_dma(bb, h, gk0, gn)
                emit_copy(bb, h, 1, copy_eng[(hi + 1) % len(copy_eng)])
                for gk0, gn in runs:
                    if gk0 + gn > 16 * h + 8:
                        emit_dma(bb, h, gk0, gn)

            prev = None
            for i, (bb, h) in enumerate(seq):
                eas[(bb, h)] = [None] * 4
                if prev is not None:
                    outsbs[prev] = outb_pool.tile(
                        [128, 16, 128], f32, tag="outsb", name="outsb"
                    )
                    for qh in range(4):
                        emit_scale_quad(*prev, qh)
                # PE: two matmul quads ahead of the B transposes
                pss = []
                for qh in range(6):
                    if qh < 4:
                        pss.append(emit_A_mm(bb, h, qh))
                        emit_A_exp(bb, h, qh, pss[qh])
                    if qh >= 2 and prev is not None:
                        emit_B_quad(*prev, qh - 2)
                for qh in range(4):
                    emit_A_sums(bb, h, qh, eas[(bb, h)][qh])
                emit_A_max(bb, h)
                emit_fin_half(bb, h)
                if h == 1:
                    emit_bg(bb)
                if prev is not None:
                    emit_B_drain(*prev)
                    for g in (0, 1):
                        psts.pop((prev[0], prev[1], g), None)
                    eas.pop(prev)
                    outsbs.pop(prev)
                prev = (bb, h)

            # epilogue: B for the final half
            outsbs[prev] = outb_pool.tile(
                [128, 16, 128], f32, tag="outsb", name="outsb"
            )
            for qh in range(4):
                emit_scale_quad(*prev, qh)
            for qh in range(4):
                emit_B_quad(*prev, qh)
            emit_B_drain(*prev)

    nc.compile()
    return nc


def _get_nc(cfg=()):
    key = ("nc",) + tuple(cfg)
    if key not in _CACHE:
        _CACHE[key] = _build(*cfg) if cfg else _build()
        try:
            from concourse.timeline_sim import TimelineSim

            _CACHE[("nc", "est")] = TimelineSim(_CACHE[key], trace=False).simulate()
        except Exception:
            pass
    if "consts" not in _CACHE:
        _CACHE["consts"] = _host_consts()
    return _CACHE[key]


def _in_maps(points, st_sizes):
    points = np.ascontiguousarray(np.asarray(points, dtype=np.float32))
    st_sizes = np.asarray(st_sizes, dtype=np.float32)
    const16, ident, identb = _CACHE["consts"]
    in_maps = []
    for c in range(NCORES):
        sl = slice(BLOC * c, BLOC * (c + 1))
        p = points[sl]  # [BLOC, N, 2]
        p = np.stack(
            [p[bb][np.argsort(p[bb, :, 1], kind="stable")] for bb in range(BLOC)]
        )
        # pad with far-away dummies so slot windows never clamp
        pp = np.empty((BLOC, NPTS, 2), np.float32)
        pp[:, :, 0] = 0.0
        pp[:, :, 1] = DUMMY_Y
        pp[:, PAD : PAD + N] = p
        xk = (pp[..., 0] - np.float32(CENTER)).astype(np.float32)
        yk = (pp[..., 1] - np.float32(CENTER)).astype(np.float32)
        r2 = (xk * xk + yk * yk).astype(np.float32)
        yhi, ylo = _split(yk)
        xhi, xlo = _split(xk)
        rhi, rlo = _split(r2)
        one = np.ones_like(xk)
        pts = np.ascontiguousarray(
            np.stack(
                [yhi, yhi, ylo, ylo, xhi, xhi, xlo, xlo, rhi, rhi, rlo, rlo,
                 one, one, one, one],
                axis=1,
            )
        )  # [BLOC, 16, NPTS]
        s = ((st_sizes[sl] * np.float32(BG_RATIO)) ** 2).astype(np.float32)
        sbg = np.ascontiguousarray(np.broadcast_to(s[None, :], (128, BLOC)))
        in_maps.append(
            {"pts": pts, "const16": const16, "sbg": sbg, "ident": ident,
             "identb": identb}
        )
    return in_maps


def _numpy_fallback(points, st_sizes):
    """Dense host computation for pathological point layouts."""
    cood = np.arange(0, C_SIZE, STRIDE, dtype=np.float64) + STRIDE / 2.0
    out = np.empty((B, N + 1, M), np.float32)
    for bb in range(B):
        x = points[bb, :, 0].astype(np.float64)
        y = points[bb, :, 1].astype(np.float64)
        xd = (x[:, None] - cood) ** 2
        yd = (y[:, None] - cood) ** 2
        dis = (yd[:, :, None] + xd[:, None, :]).reshape(N, M)
        mind = np.clip(dis.min(axis=0), 0.0, None)
        bg = (float(st_sizes[bb]) * BG_RATIO) ** 2 / (mind + EPS)
        logits = np.concatenate([dis, bg[None]], axis=0) * (-1.0 / 128.0)
        e = np.exp(logits - logits.max(axis=0))
        out[bb] = (e / e.sum(axis=0)).astype(np.float32)
    return out


def _run(points, st_sizes, trace=False):
    from concourse.bass_utils import run_bass_kernel_spmd

    points = np.ascontiguousarray(np.asarray(points, dtype=np.float32))
    ow_t = _fit_windows(points)
    if ow_t is None:
        return _numpy_fallback(points, np.asarray(st_sizes)), None
    canon = tuple(_windows()[1])
    nc = _get_nc((("act", "act"), None if ow_t == canon else ow_t))
    _CACHE["last_est"] = _CACHE.get(("nc", "est"))
    res = run_bass_kernel_spmd(
        nc, _in_maps(points, st_sizes), core_ids=list(range(NCORES)),
        trace=trace,
    )
    out = np.concatenate([r["out"] for r in res.results], axis=0)
    # rows are in y-sorted order on device; scatter back to input order
    perm = np.argsort(points[..., 1], axis=1, kind="stable")  # [B, N]
    full = np.zeros_like(out)
    for bb in range(B):
        full[bb, perm[bb], :] = out[bb, :N, :]
        full[bb, N, :] = out[bb, N, :]
    return full, res


def kernel(points, st_sizes):
    out, _ = _run(points, st_sizes, trace=False)
    return out


def kernel_profiled(points, st_sizes):
    """Returns (out, BassKernelResults) with exec_time_ns populated."""
    return _run(points, st_sizes, trace=True)


# revision 44
# speedup vs baseline: 1.7860x; 1.0267x over previous
"""Trainium2 Bass kernel for nn_Post_Prob (segment_reduce).

Reference computation, per image b (N=512 points, M=64*64=4096 pixels):
    dis[p, ij]  = (y_p - cood_i)^2 + (x_p - cood_j)^2
    min_dis[ij] = relu(min_p dis[p, ij])
    bg[ij]      = (0.15 * st_b)^2 / (min_dis + 1e-5)
    out         = softmax over the 513 rows of [-dis/128 ; -bg/128]

Sharding: data-parallel over the batch axis, 16 images -> 8 cores x 2.

Slot-window design: points are y-sorted on the host, so point index
("slot") tracks y within ~27 px on this data.  Every per-pixel-chunk
computation then touches only a STATIC contiguous slot window:
  - matmul window: 256 slots centred on the chunk's y (padded pts array
    keeps it unclamped; fp32r needs >=256 output columns for 1 cyc/row)
  - work window 128 slots: exp (bf16) / per-quad sum + max reduces /
    per-quad broadcast scale run over [128 px, 128 slot] tiles
  - min_dis = -128*ln(max of exp) -- turns the PSUM min-reduce into a
    cheap SBUF max-reduce and two small fin ops
  - output: the scaled exp tile is PE-transposed (bf16, 1 cyc/row) to
    [slot, px] PSUM, copied to SBUF by the otherwise-idle GpSimd engine,
    and DMA'd with batched 3D APs (rows advance 16/chunk, 512B rows).
Values outside the windows are provably below ~2e-4 of the softmax
scale (the background term keeps Z >= 0.55); bf16 adds ~0.4% relative
noise -- both well inside the 2e-2 gate.
"""

import numpy as np

SIGMA = 8.0
C_SIZE = 512
STRIDE = 8
BG_RATIO = 0.15
EPS = 1e-5
B, N = 16, 512
C = C_SIZE // STRIDE  # 64
M = C * C  # 4096
NCORES = 8
BLOC = B // NCORES  # 2 images per core
INV = -1.0 / (2.0 * SIGMA * SIGMA)  # -1/128
CENTER = 256.0
NCHUNK = M // 128  # 32 pixel chunks (2 grid rows each)
PAD = 128  # slot padding each side of the 512 real slots
NPTS = N + 2 * PAD  # 768
DUMMY_Y = 8000.0
W0 = 112  # write/work window (slots)
T_WRITE = 30.0  # |dy| that must be inside the write window
T_SUM = 30.0  # |dy| that must be inside the work window

_CACHE = {}


def _split(v, bits=11):
    """v = hi + lo with hi keeping `bits` explicit mantissa bits."""
    u = np.ascontiguousarray(v, dtype=np.float32).view(np.uint32)
    hi = (u & np.uint32((0xFFFFFFFF << (23 - bits)) & 0xFFFFFFFF)).view(np.float32)
    lo = (v - hi).astype(np.float32)
    return hi, lo


def _host_consts():
    import ml_dtypes

    cood = (np.arange(0, C_SIZE, STRIDE, dtype=np.float32) + STRIDE / 2.0).astype(
        np.float32
    )
    cc = cood - np.float32(CENTER)
    ci = np.repeat(cc, C).astype(np.float32)  # i (y) varies slow over ij
    cj = np.tile(cc, C).astype(np.float32)  # j (x) varies fast
    c2 = (ci * ci + cj * cj).astype(np.float32)
    ones = np.ones(M, np.float32)
    zero = np.zeros(M, np.float32)
    ahi, alo = _split(-2.0 * ci)
    bhi, blo = _split(-2.0 * cj)
    chi, clo = _split(c2)
    const16 = np.stack(
        [
            ahi, alo, ahi, alo,       # k=0: * (y'hi, y'hi, y'lo, y'lo)
            bhi, blo, bhi, blo,       # k=1: * (x'hi, x'hi, x'lo, x'lo)
            ones, zero, ones, zero,   # k=2: * (r2hi, r2hi, r2lo, r2lo)
            chi, clo, zero, zero,     # k=3: * (1, 1, 1, 1)
        ]
    ).astype(np.float32)  # [16, M]
    ident = np.eye(128, dtype=np.float32)
    identb = np.eye(128).astype(ml_dtypes.bfloat16)
    return const16, ident, identb


def _windows(ow=None):
    """Per-chunk static slot windows (padded coords for matmul/work)."""
    o_mm = [16 * k + 8 for k in range(NCHUNK)]  # padded start of 256-window
    if ow is None:
        ow = [min(max(16 * k + 8 - W0 // 2, 0), N - W0) for k in range(NCHUNK)]
    w_off = [ow[k] + PAD - o_mm[k] for k in range(NCHUNK)]  # within [0,256-W0]
    return o_mm, list(ow), w_off


def _fit_windows(points):
    """Choose per-chunk window starts covering this data's candidate
    slots.  Returns (ow tuple, grouping) or None if infeasible.  Prefers
    the canonical stride-16 layout; falls back to per-chunk shifts with
    degraded DMA batching."""
    canon = [min(max(16 * k + 8 - W0 // 2, 0), N - W0) for k in range(NCHUNK)]
    lo = [0] * NCHUNK
    hi = [-1] * NCHUNK
    for bb in range(points.shape[0]):
        y = np.sort(points[bb, :, 1])
        for k in range(NCHUNK):
            c = 16 * k + 8
            m = np.abs(y - c) <= T_WRITE
            if m.any():
                idx = np.nonzero(m)[0]
                lo[k] = min(lo[k], int(idx[0])) if hi[k] >= 0 else int(idx[0])
                hi[k] = max(hi[k], int(idx[-1]))
    ow = []
    for k in range(NCHUNK):
        if hi[k] < 0:
            ow.append(canon[k])
            continue
        if hi[k] - lo[k] >= W0:
            return None  # window cannot cover; numpy fallback
        o = min(max(canon[k], hi[k] - W0 + 1), lo[k])
        o = min(max(o, 0), N - W0)
        if o > lo[k] or o + W0 <= hi[k]:
            return None
        # matmul 256-window must contain the work window
        if not (0 <= o + PAD - (16 * k + 8) <= 256 - W0):
            return None
        ow.append(o)
    return tuple(ow)


def _dma_groups(ow):
    """Split each half's 16 chunks into maximal uniform-stride runs."""
    groups = {}
    for h in (0, 1):
        k0, k1 = 16 * h, 16 * h + 16
        runs = []
        s = k0
        while s < k1:
            cap = k0 + 8 if s < k0 + 8 else k1  # break at copy boundary
            e = s + 1
            if e < cap:
                d = ow[e] - ow[e - 1]
                while e < cap and ow[e] - ow[e - 1] == d:
                    e += 1
            runs.append((s, e - s))
            s = e
        groups[h] = runs
    return groups


def _quad_spans(w_off):
    """Per quad: (base, width) covering its 4 chunks' work windows."""
    spans = []
    for q in range(NCHUNK // 4):
        offs = w_off[4 * q : 4 * q + 4]
        lo, hi = min(offs), max(offs) + W0
        spans.append((lo, hi - lo))
    return spans


def _force_combined_act_table(arch="gen3"):
    """Keep exp+ln+copy in one activation table (avoids ~2.7us reloads)."""
    import concourse.hw_specs as hw_specs

    tabs = hw_specs.get_activation_tables(arch)
    keep = "natural_log_exp_and_others"
    if keep in tabs:
        for name, s in tabs.items():
            if name != keep:
                s.clear()


def _build(copy_eng=("act", "act"), ow_t=None, pst_bufs=2, warm_n=0, ea_bufs=12, outb_bufs=3):
    import concourse.bacc as bacc
    import concourse.tile as tile
    import concourse.mybir as mybir
    from concourse.ap import AP

    _force_combined_act_table()

    f32 = mybir.dt.float32
    f32r = mybir.dt.float32r
    bf16 = mybir.dt.bfloat16
    AF = mybir.ActivationFunctionType
    OP = mybir.AluOpType
    AX = mybir.AxisListType

    o_mm, ow, w_off = _windows(ow_t)
    spans = _quad_spans(w_off)
    dgroups = _dma_groups(ow)
    NQ = NCHUNK // 4  # 8 quads per image

    def _ap(base_ap, extra_off, dims):
        """Custom-stride sub-AP of an existing AP (element units)."""
        pdim = [int(base_ap.ap[0][0]), int(base_ap.ap[0][1])]
        return AP(base_ap.tensor, base_ap.offset + extra_off, [pdim] + dims)

    nc = bacc.Bacc("TRN2", target_bir_lowering=False, debug=False, num_devices=NCORES)

    pts_d = nc.dram_tensor("pts", [BLOC, 16, NPTS], f32r, kind="ExternalInput")
    const16_d = nc.dram_tensor("const16", [16, M], f32r, kind="ExternalInput")
    sbg_d = nc.dram_tensor("sbg", [128, BLOC], f32, kind="ExternalInput")
    id_d = nc.dram_tensor("ident", [128, 128], f32, kind="ExternalInput")
    idb_d = nc.dram_tensor("identb", [128, 128], bf16, kind="ExternalInput")
    out_d = nc.dram_tensor("out", [BLOC, N + 1, M], f32, kind="ExternalOutput")

    EWM = max(w for _, w in spans)  # 176 on this layout

    with tile.TileContext(nc) as tc:
        with (
            tc.tile_pool(name="singles", bufs=1) as singles,
            tc.tile_pool(name="psA", bufs=2, space="PSUM") as psA_pool,
            tc.tile_pool(name="pst", bufs=pst_bufs, space="PSUM") as pst_pool,
            tc.tile_pool(name="warmps", bufs=1, space="PSUM") as warm_pool,
            tc.tile_pool(name="ea", bufs=ea_bufs) as ea_pool,
            tc.tile_pool(name="outb", bufs=outb_bufs) as outb_pool,
            tc.tile_pool(name="cols", bufs=1) as cols_pool,
            tc.tile_pool(name="fin", bufs=2) as fin_pool,
        ):
            # input loads: first half of image 0 is on the critical path
            const16_t = singles.tile([16, M], f32r)
            pts_tiles = []
            for bb in range(BLOC):
                pt = singles.tile([16, NPTS], f32r, tag=f"pts{bb}")
                pts_tiles.append(pt)
            # first A-quad needs pts0 + const16 cols 0:512 only; ACT ring
            # carries just pts0 so the first exp isn't queued behind bulk,
            # Pool SWDGE takes the small constants (Pool idles early on)
            nc.sync.dma_start(const16_t[:, 0:512], const16_d[:, 0:512])
            nc.gpsimd.dma_start(pts_tiles[0][:], pts_d[0])
            idb_t = singles.tile([128, 128], bf16)
            nc.scalar.dma_start(idb_t[:], idb_d[:])
            nc.sync.dma_start(const16_t[:, 512:2048], const16_d[:, 512:2048])
            id_t = singles.tile([128, 128], f32)
            nc.gpsimd.dma_start(id_t[:], id_d[:])
            sbg_t = singles.tile([128, BLOC], f32)
            nc.gpsimd.dma_start(sbg_t[:], sbg_d[:])
            nc.scalar.dma_start(pts_tiles[1][:], pts_d[1])
            nc.sync.dma_start(const16_t[:, 2048:], const16_d[:, 2048:])

            cols = {}
            for bb in range(BLOC):
                cols[bb] = {}
                for nm, dt_ in (("mx", bf16), ("sm", f32), ("rz", f32),
                                ("bgp", f32)):
                    cols[bb][nm] = cols_pool.tile(
                        [128, NCHUNK], dt_, tag=f"{nm}{bb}", name=f"{nm}{bb}"
                    )

            eas = {}  # (bb, h) -> list of 4 ea tiles
            psts = {}  # (bb, h, g) -> pst tile

            def emit_A_mm(bb, h, qh):
                q = 4 * h + qh
                ps = psA_pool.tile([128, 4, 256], f32, tag="psA")
                for j in range(4):
                    k = 4 * q + j
                    nc.tensor.matmul(
                        ps[:, j, :],
                        const16_t[:, k * 128 : (k + 1) * 128],
                        pts_tiles[bb][:, o_mm[k] : o_mm[k] + 256],
                        start=True,
                        stop=True,
                    )
                return ps

            def emit_A_exp(bb, h, qh, ps):
                q = 4 * h + qh
                offs = w_off[4 * q : 4 * q + 4]
                dq = offs[1] - offs[0]  # 0 interior, -16 at the edges
                ea = ea_pool.tile([128, 4, W0], bf16, tag="ea", name="ea")
                if all(offs[j] == offs[0] + dq * j for j in range(4)):
                    src = _ap(ps[:], offs[0], [[256 + dq, 4], [1, W0]])
                    nc.scalar.activation(
                        ea[:], src, AF.Exp, bias=0.0, scale=INV
                    )
                else:  # fitted windows: per-chunk offsets, unbatched
                    for j in range(4):
                        nc.scalar.activation(
                            ea[:, j, :], ps[:, j, offs[j] : offs[j] + W0],
                            AF.Exp, bias=0.0, scale=INV,
                        )
                eas[(bb, h)][qh] = ea
                return ea

            def emit_A_sums(bb, h, qh, ea):
                """Per-chunk sum-of-exp via 4x-mode tensor_scalar accum."""
                q = 4 * h + qh
                sm = cols[bb]["sm"]
                for j in range(4):
                    k = 4 * q + j
                    nc.vector.tensor_scalar(
                        ea[:, j, :], ea[:, j, :], 1.0, 0.0,
                        op0=OP.mult, op1=OP.add, accum_out=sm[:, k : k + 1],
                    )

            def emit_A_max(bb, h):
                """Half-level window max: two 2x tt-max halvings + reduce."""
                cc = cols[bb]
                qs = eas[(bb, h)]
                hw_, qw = W0 // 2, W0 // 4
                t1 = fin_pool.tile([128, 4, 4, hw_], bf16, tag="t1", name="t1")
                for qh in range(4):
                    ea = qs[qh]
                    nc.vector.tensor_tensor(
                        t1[:, qh, :, :], ea[:, :, 0:hw_], ea[:, :, hw_ : 2 * hw_],
                        op=OP.max,
                    )
                nc.vector.tensor_tensor(
                    t1[:, :, :, 0:qw], t1[:, :, :, 0:qw],
                    t1[:, :, :, qw : 2 * qw], op=OP.max,
                )
                nc.vector.tensor_reduce(
                    cc["mx"][:, 16 * h : 16 * h + 16],
                    t1[:, :, :, 0:qw], axis=AX.X, op=OP.max,
                )

            def emit_scale_quad(bb, h, qh):
                q = 4 * h + qh
                ea = eas[(bb, h)][qh]
                rz = cols[bb]["rz"]
                for j in range(4):
                    k = 4 * q + j
                    nc.vector.tensor_scalar_mul(
                        ea[:, j, :], ea[:, j, :], rz[:, k : k + 1]
                    )

            def emit_T_quad(bb, h, qh):
                ea = eas[(bb, h)][qh]
                g = qh // 2
                if qh % 2 == 0:
                    psts[(bb, h, g)] = pst_pool.tile([128, 8, 128], bf16, tag="pst", name="pst")
                ps = psts[(bb, h, g)]
                for j in range(4):
                    nc.tensor.transpose(
                        ps[:W0, 4 * (qh % 2) + j, :], ea[:, j, :], idb_t[:]
                    )

            def emit_copy(bb, h, g, eng):
                """copy a transposed 8-chunk group psum->sbuf (+f32 convert).
                GPSIMD cannot read PSUM, so only ACT/DVE are legal here."""
                outsb = outsbs[(bb, h)]
                ps = psts[(bb, h, g)]
                dst = outsb[:W0, 8 * g : 8 * g + 8, :]
                srcg = ps[:W0, :, :]
                if eng == "act":
                    nc.scalar.copy(dst, srcg)
                else:
                    nc.vector.tensor_copy(dst, srcg)

            def emit_dma(bb, h, gk0, gn):
                """output rows for gn chunks: [W0 slots, gn chunks, 128 px]."""
                outsb = outsbs[(bb, h)]
                row0 = ow[gk0]
                drow = ow[gk0 + 1] - ow[gk0] if gn > 1 else 0
                bs = out_d[bb]
                dst = AP(
                    bs.tensor,
                    bs.offset + row0 * M + gk0 * 128,
                    [[M, W0], [drow * M + 128, gn], [1, 128]],
                )
                c0 = gk0 - 16 * h
                nc.sync.dma_start(dst, outsb[:W0, c0 : c0 + gn, :])

            def emit_fin_half(bb, h):
                """chunks 16h..16h+15: bg row segment + 1/Z per pixel."""
                cc = cols[bb]
                s = slice(16 * h, 16 * (h + 1))
                lnm = fin_pool.tile([128, 16], f32, tag="lnm")
                nc.scalar.activation(lnm[:], cc["mx"][:, s], AF.Ln, bias=0.0, scale=1.0)
                # min = -128 * clamp(ln(max), -90, 0); then *(-128) + eps
                nc.vector.tensor_scalar(
                    lnm[:], lnm[:], -90.0, 0.0, op0=OP.max, op1=OP.min
                )
                tmp = fin_pool.tile([128, 16], f32, tag="tmp")
                nc.vector.tensor_scalar(
                    tmp[:], lnm[:], -(2.0 * SIGMA * SIGMA), EPS,
                    op0=OP.mult, op1=OP.add,
                )
                rmin = fin_pool.tile([128, 16], f32, tag="rmin")
                nc.vector.reciprocal(rmin[:], tmp[:])
                bgd = fin_pool.tile([128, 16], f32, tag="bgd")
                nc.vector.tensor_scalar_mul(bgd[:], rmin[:], sbg_t[:, bb : bb + 1])
                ebg = fin_pool.tile([128, 16], f32, tag="ebg")
                nc.scalar.activation(ebg[:], bgd[:], AF.Exp, bias=0.0, scale=INV)
                stot = fin_pool.tile([128, 16], f32, tag="stot")
                nc.vector.tensor_tensor(stot[:], cc["sm"][:, s], ebg[:], op=OP.add)
                nc.vector.reciprocal(cc["rz"][:, s], stot[:])
                nc.vector.tensor_tensor(
                    cc["bgp"][:, s], ebg[:], cc["rz"][:, s], op=OP.mult
                )

            def emit_bg(bb):
                """background row: transpose bg probs, copy, one 16KB DMA."""
                ps = pst_pool.tile([128, 8, 128], bf16, tag="pst", name="pst")
                psf = ps[:32, 0:2, :].bitcast(f32)
                nc.tensor.transpose(psf, cols[bb]["bgp"][:], id_t[:])
                tsb = fin_pool.tile([32, 128], f32, tag="tsb")
                nc.scalar.copy(tsb[:], psf)
                nc.sync.dma_start(
                    out_d[bb, N, :].rearrange("(k q) -> k q", q=128), tsb[:]
                )

            # warm the activation table at t=0, and ramp the PE p-state
            # with a ~3us dummy matmul chain while the inputs stream in
            warm = fin_pool.tile([1, 1], f32, tag="warm")
            nc.vector.memset(warm[:], 0.0)
            nc.scalar.activation(warm[:], warm[:], AF.Exp, bias=0.0, scale=1.0)
            wsrc = singles.tile([16, 128], f32)
            nc.vector.memset(wsrc[:], 0.0)
            wps = warm_pool.tile([128, 128], f32)
            for _ in range(warm_n):
                nc.tensor.matmul(wps[:], wsrc[:], wsrc[:, 0:128], start=True,
                                 stop=True)

            seq = [(bb, h) for bb in range(BLOC) for h in range(2)]
            outsbs = {}

            def emit_B_quad(bb, h, qh):
                emit_T_quad(bb, h, qh)

            def emit_B_drain(bb, h, last=False):
                """copies + DMAs; emitted late so ACT exps aren't blocked.
                For the final half, drain per quad so the serial DMA burst
                starts as early as possible."""
                hi = 2 * (2 * bb + h)
                runs = dgroups[h]
                if last:
                    for qh in range(4):
                        g, j = divmod(qh, 2)
                        ps = psts[(bb, h, g)]
                        outsb = outsbs[(bb, h)]
                        nc.scalar.copy(
                            outsb[:W0, 4 * qh : 4 * qh + 4, :],
                            ps[:W0, 4 * j : 4 * j + 4, :],
                        )
                        for gk0, gn in runs:
                            if 16 * h + 4 * qh <= gk0 and gk0 + gn <= 16 * h + 4 * qh + 4:
                                emit_dma(bb, h, gk0, gn)
                    return
                emit_copy(bb, h, 0, copy_eng[hi % len(copy_eng)])
                for gk0, gn in runs:
                    if gk0 + gn <= 16 * h + 8:
                        emit_dma(bb, h, gk0, gn)
                emit_copy(bb, h, 1, copy_eng[(hi + 1) % len(copy_eng)])
                for gk0, gn in runs:
                    if gk0 + gn > 16 * h + 8:
                        emit_dma(bb, h, gk0, gn)

            prev = None
            for i, (bb, h) in enumerate(seq):
                eas[(bb, h)] = [None] * 4
                if prev is not None:
                    outsbs[prev] = outb_pool.tile(
                        [128, 16, 128], f32, tag="outsb", name="outsb"
                    )
                    for qh in range(4):
                        emit_scale_quad(*prev, qh)
                # PE: two matmul quads ahead of the B transposes
                pss = []
                for qh in range(6):
                    if qh < 4:
                        pss.append(emit_A_mm(bb, h, qh))
                        emit_A_exp(bb, h, qh, pss[qh])
                    if qh >= 2 and prev is not None:
                        emit_B_quad(*prev, qh - 2)
                for qh in range(4):
                    emit_A_sums(bb, h, qh, eas[(bb, h)][qh])
                emit_A_max(bb, h)
                emit_fin_half(bb, h)
                if h == 1:
                    emit_bg(bb)
                if prev is not None:
                    emit_B_drain(*prev)
                    for g in (0, 1):
                        psts.pop((prev[0], prev[1], g), None)
                    eas.pop(prev)
                    outsbs.pop(prev)
                prev = (bb, h)

            # epilogue: B for the final half
            outsbs[prev] = outb_pool.tile(
                [128, 16, 128], f32, tag="outsb", name="outsb"
            )
            for qh in range(4):
                emit_scale_quad(*prev, qh)
            for qh in range(4):
                emit_B_quad(*prev, qh)
            emit_B_drain(*prev)

    nc.compile()
    return nc


def _get_nc(cfg=()):
    key = ("nc",) + tuple(cfg)
    if key not in _CACHE:
        _CACHE[key] = _build(*cfg) if cfg else _build()
        try:
            from concourse.timeline_sim import TimelineSim

            _CACHE[("nc", "est")] = TimelineSim(_CACHE[key], trace=False).simulate()
        except Exception:
            pass
    if "consts" not in _CACHE:
        _CACHE["consts"] = _host_consts()
    return _CACHE[key]


def _in_maps(points, st_sizes):
    points = np.ascontiguousarray(np.asarray(points, dtype=np.float32))
    st_sizes = np.asarray(st_sizes, dtype=np.float32)
    const16, ident, identb = _CACHE["consts"]
    in_maps = []
    for c in range(NCORES):
        sl = slice(BLOC * c, BLOC * (c + 1))
        p = points[sl]  # [BLOC, N, 2]
        p = np.stack(
            [p[bb][np.argsort(p[bb, :, 1], kind="stable")] for bb in range(BLOC)]
        )
        # pad with far-away dummies so slot windows never clamp
        pp = np.empty((BLOC, NPTS, 2), np.float32)
        pp[:, :, 0] = 0.0
        pp[:, :, 1] = DUMMY_Y
        pp[:, PAD : PAD + N] = p
        xk = (pp[..., 0] - np.float32(CENTER)).astype(np.float32)
        yk = (pp[..., 1] - np.float32(CENTER)).astype(np.float32)
        r2 = (xk * xk + yk * yk).astype(np.float32)
        yhi, ylo = _split(yk)
        xhi, xlo = _split(xk)
        rhi, rlo = _split(r2)
        one = np.ones_like(xk)
        pts = np.ascontiguousarray(
            np.stack(
                [yhi, yhi, ylo, ylo, xhi, xhi, xlo, xlo, rhi, rhi, rlo, rlo,
                 one, one, one, one],
                axis=1,
            )
        )  # [BLOC, 16, NPTS]
        s = ((st_sizes[sl] * np.float32(BG_RATIO)) ** 2).astype(np.float32)
        sbg = np.ascontiguousarray(np.broadcast_to(s[None, :], (128, BLOC)))
        in_maps.append(
            {"pts": pts, "const16": const16, "sbg": sbg, "ident": ident,
             "identb": identb}
        )
    return in_maps


def _numpy_fallback(points, st_sizes):
    """Dense host computation for pathological point layouts."""
    cood = np.arange(0, C_SIZE, STRIDE, dtype=np.float64) + STRIDE / 2.0
    out = np.empty((B, N + 1, M), np.float32)
    for bb in range(B):
        x = points[bb, :, 0].astype(np.float64)
        y = points[bb, :, 1].astype(np.float64)
        xd = (x[:, None] - cood) ** 2
        yd = (y[:, None] - cood) ** 2
        dis = (yd[:, :, None] + xd[:, None, :]).reshape(N, M)
        mind = np.clip(dis.min(axis=0), 0.0, None)
        bg = (float(st_sizes[bb]) * BG_RATIO) ** 2 / (mind + EPS)
        logits = np.concatenate([dis, bg[None]], axis=0) * (-1.0 / 128.0)
        e = np.exp(logits - logits.max(axis=0))
        out[bb] = (e / e.sum(axis=0)).astype(np.float32)
    return out


def _run(points, st_sizes, trace=False):
    from concourse.bass_utils import run_bass_kernel_spmd

    points = np.ascontiguousarray(np.asarray(points, dtype=np.float32))
    ow_t = _fit_windows(points)
    if ow_t is None:
        return _numpy_fallback(points, np.asarray(st_sizes)), None
    canon = tuple(_windows()[1])
    nc = _get_nc((("act", "act"), None if ow_t == canon else ow_t))
    _CACHE["last_est"] = _CACHE.get(("nc", "est"))
    res = run_bass_kernel_spmd(
        nc, _in_maps(points, st_sizes), core_ids=list(range(NCORES)),
        trace=trace,
    )
    out = np.concatenate([r["out"] for r in res.results], axis=0)
    # rows are in y-sorted order on device; scatter back to input order
    perm = np.argsort(points[..., 1], axis=1, kind="stable")  # [B, N]
    full = np.zeros_like(out)
    for bb in range(B):
        full[bb, perm[bb], :] = out[bb, :N, :]
        full[bb, N, :] = out[bb, N, :]
    return full, res


def kernel(points, st_sizes):
    out, _ = _run(points, st_sizes, trace=False)
    return out


def kernel_profiled(points, st_sizes):
    """Returns (out, BassKernelResults) with exec_time_ns populated."""
    return _run(points, st_sizes, trace=True)
